# revision 1
# baseline (speedup 1.0000x reference)
"""Bass/Tile kernel for nn_CNN2: lagged cross-correlation + mean/var + tiny CNN head.

Sharding: interleaved lags across 8 cores. Core m computes lags
d = 128h + 32*k4 + 4m + (3-d1), h in [0,16), k4 in [0,4), d1 in [0,4).
The 4m offset is baked into per-core input data placement (PXR); the program is
identical across cores (SPMD).
"""
import numpy as np
import ml_dtypes

import concourse.bass as bass
import concourse.bacc as bacc
import concourse.tile as tile
from concourse import mybir

bf16 = ml_dtypes.bfloat16
FP32 = mybir.dt.float32
BF16 = mybir.dt.bfloat16

T = 2048
ROW = 30
NC = 8
OL = 4            # lhsT frame offset
BM = 128          # buffer lead margin
NCH = 18          # YLB/YRB chunks
WBUF = 128 * NCH  # 2304
NCC = 17          # contraction chunks
LEAD = 3          # lead zero chunks in YR copies
RHO = (3, 35, 67, 99)
NLAG = 2 * T - 1  # 4095

INPUT_SPECS = [
    ("pxl", [ROW, WBUF], BF16), ("pxr", [ROW, WBUF], BF16),
    ("wt", [ROW, ROW], BF16),
    ("blockind", [120, 4], BF16), ("diagmask", [120, 480], FP32),
    ("ident", [ROW, ROW], FP32), ("c0mask", [ROW, 1], FP32),
    ("cw1", [32, 8], FP32), ("cb1", [8, 1], FP32),
    ("cw2", [32, 16], FP32), ("cb2", [16, 1], FP32),
    ("fwt", [16, 2], FP32), ("fb", [1, 2], FP32),
    ("bindT", [4, 120], FP32),
]


# ---------------------------------------------------------------- host prep
def host_inputs(x, W, conv1_w, conv1_b, conv2_w, conv2_b, fc_w, fc_b):
    """Returns per-core input maps (program constants + per-core data)."""
    x = np.asarray(x, np.float32)
    W = np.asarray(W, np.float32)

    def px(off):
        p = np.zeros((ROW, WBUF), bf16)
        w0 = BM + off
        n = min(T, WBUF - w0)
        p[:, w0:w0 + n] = x[:, :n].astype(bf16)
        return p

    blockind = np.zeros((120, 4), bf16)
    for d1 in range(4):
        blockind[d1 * 30:d1 * 30 + 30, d1] = 1.0

    diagmask = np.zeros((120, 480), np.float32)
    for d1 in range(4):
        for b2 in range(16):
            for i in range(ROW):
                diagmask[d1 * 30 + i, b2 * 30 + i] = 1.0

    shared = {
        "pxl": px(OL),
        "wt": np.ascontiguousarray(W.T).astype(bf16),   # rhs[k,i] = W[i,k]
        "blockind": blockind,
        "diagmask": diagmask,
        "ident": np.eye(ROW, dtype=np.float32),
        "cw1": np.ascontiguousarray(np.asarray(conv1_w, np.float32).reshape(8, 32).T),
        "cb1": np.asarray(conv1_b, np.float32).reshape(8, 1),
        "cw2": np.ascontiguousarray(
            np.asarray(conv2_w, np.float32).transpose(2, 3, 1, 0).reshape(32, 16)),
        "cb2": np.asarray(conv2_b, np.float32).reshape(16, 1),
        "fwt": np.ascontiguousarray(np.asarray(fc_w, np.float32).T),   # [16,2]
        "fb": np.asarray(fc_b, np.float32).reshape(1, 2),
        "bindT": np.ascontiguousarray(blockind.astype(np.float32).T),  # [4,120]
    }
    per_core = []
    for m in range(NC):
        d = dict(shared)
        d["pxr"] = px(OL + 4 * m)
        d["c0mask"] = np.full((ROW, 1), 1.0 if m == 0 else 0.0, np.float32)
        per_core.append(d)
    return per_core


def mk(t, off, dims):
    """AP on tile t with explicit free dims; partition pair preserved from t[:]."""
    ap = t[:]
    return bass.AP(ap.tensor, off, [list(ap.ap[0])] + [list(d) for d in dims])


# ---------------------------------------------------------------- kernel
def build_nc():
    nc = bacc.Bacc("TRN2", target_bir_lowering=False, debug=False, num_devices=NC)
    din = {}
    for name, shape, dt in INPUT_SPECS:
        din[name] = nc.dram_tensor(name, shape, dt, kind="ExternalInput").ap()
    out_d = nc.dram_tensor("out", [1, 2], FP32, kind="ExternalOutput").ap()
    with tile.TileContext(nc) as tc:
        _body(tc, din, out_d)
    nc.compile()
    return nc


def _body(tc, din, out_d):
    nc = tc.nc
    AT = mybir.AluOpType
    AX = mybir.AxisListType
    AF = mybir.ActivationFunctionType

    from contextlib import ExitStack
    ctx = ExitStack()
    with ctx:
        consts = ctx.enter_context(tc.tile_pool(name="consts", bufs=1))
        base_p = ctx.enter_context(tc.tile_pool(name="base", bufs=1))
        copies_p = ctx.enter_context(tc.tile_pool(name="copies", bufs=1))
        wf_psum = ctx.enter_context(tc.tile_pool(name="wfpsum", bufs=2, space="PSUM"))
        mm_psum = ctx.enter_context(tc.tile_pool(name="mmpsum", bufs=2, space="PSUM"))
        tr_psum = ctx.enter_context(tc.tile_pool(name="trpsum", bufs=2, space="PSUM"))
        work = ctx.enter_context(tc.tile_pool(name="work", bufs=2))
        accs = ctx.enter_context(tc.tile_pool(name="accs", bufs=1))
        dram = ctx.enter_context(tc.tile_pool(name="dram", bufs=1, space="DRAM"))
        head_psum = ctx.enter_context(tc.tile_pool(name="headpsum", bufs=2, space="PSUM"))
        headp = ctx.enter_context(tc.tile_pool(name="head", bufs=1))

        # ---- load inputs to SBUF
        sb = {}
        for name, shape, dt in INPUT_SPECS:
            t = consts.tile(shape, dt, tag=name)
            nc.sync.dma_start(t[:], din[name][:])
            sb[name] = t

        # ---- W-fold: build YLB/YRB [128, NCH*30] bf16 (cols = c*30 + i)
        ylb = base_p.tile([128, NCH * ROW], BF16, tag="ylb")
        yrb = base_p.tile([128, NCH * ROW], BF16, tag="yrb")
        for src, dst in ((sb["pxl"], ylb), (sb["pxr"], yrb)):
            for c in range(NCH):
                ps = wf_psum.tile([128, ROW], FP32, tag="wf")
                nc.tensor.matmul(ps[:], src[:, 128 * c:128 * c + 128], sb["wt"][:],
                                 start=True, stop=True)
                nc.vector.tensor_copy(dst[:, c * ROW:(c + 1) * ROW], ps[:])

        # ---- shifted copies (chunk-major: ylc cols = (c:17, d1:4, i:30))
        ylc = copies_p.tile([128, NCC * 120], BF16, tag="ylc")
        for d1 in range(4):
            if d1 == 0:
                dstA = mk(ylc, d1 * ROW, [[120, NCC], [1, ROW]])
                srcA = mk(ylb, ROW, [[ROW, NCC], [1, ROW]])
                nc.sync.dma_start(dstA, srcA)
            else:
                ylc_s = ylc[d1:128, :]
                dstA = bass.AP(ylc_s.tensor, ylc_s.offset + d1 * ROW,
                               [list(ylc_s.ap[0])] + [[120, NCC], [1, ROW]])
                ylb_s = ylb[0:128 - d1, :]
                srcA = bass.AP(ylb_s.tensor, ylb_s.offset + ROW,
                               [list(ylb_s.ap[0])] + [[ROW, NCC], [1, ROW]])
                nc.sync.dma_start(dstA, srcA)
                ylc_t = ylc[0:d1, :]
                dstB = bass.AP(ylc_t.tensor, ylc_t.offset + d1 * ROW,
                               [list(ylc_t.ap[0])] + [[120, NCC], [1, ROW]])
                ylb_t = ylb[128 - d1:128, :]
                srcB = bass.AP(ylb_t.tensor, ylb_t.offset,
                               [list(ylb_t.ap[0])] + [[ROW, NCC], [1, ROW]])
                nc.sync.dma_start(dstB, srcB)
        # yrc cols = (cc:20, v:4, i:30); lead 3 chunks zero
        yrc = copies_p.tile([128, (LEAD + NCC) * 120], BF16, tag="yrc")
        nc.vector.memset(yrc[:, 0:LEAD * 120], 0.0)
        for v, rho in enumerate(RHO):
            yrc_s = yrc[rho:128, :]
            dstA = bass.AP(yrc_s.tensor, yrc_s.offset + LEAD * 120 + v * ROW,
                           [list(yrc_s.ap[0])] + [[120, NCC], [1, ROW]])
            yrb_s = yrb[0:128 - rho, :]
            srcA = bass.AP(yrb_s.tensor, yrb_s.offset + ROW,
                           [list(yrb_s.ap[0])] + [[ROW, NCC], [1, ROW]])
            nc.sync.dma_start(dstA, srcA)
            yrc_t = yrc[0:rho, :]
            dstB = bass.AP(yrc_t.tensor, yrc_t.offset + LEAD * 120 + v * ROW,
                           [list(yrc_t.ap[0])] + [[120, NCC], [1, ROW]])
            yrb_t = yrb[128 - rho:128, :]
            srcB = bass.AP(yrb_t.tensor, yrb_t.offset,
                           [list(yrb_t.ap[0])] + [[ROW, NCC], [1, ROW]])
            nc.sync.dma_start(dstB, srcB)

        # ---- main matmul groups + trace + scaled accumulation
        acc1 = accs.tile([120, ROW], FP32, tag="acc1")
        acc2 = accs.tile([120, ROW], FP32, tag="acc2")
        z0keep = accs.tile([ROW, ROW], FP32, tag="z0keep")

        for g in range(4):
            ps = mm_psum.tile([120, 480], FP32, tag="mm")
            first = True
            for c in range(4 * g, NCC):
                lhsT = mk(ylc, c * 120, [[1, 120]])
                rhs = mk(yrc, (LEAD + c - 4 * g) * 120,
                         [[ROW, 4], [-120, 4], [1, ROW]])
                nc.tensor.matmul(ps[:], lhsT, rhs, start=first, stop=(c == NCC - 1))
                first = False
            # traces: D = psum * diagmask (bf16), ones-mm, reduce inner j
            D = work.tile([120, 480], BF16, tag="D")
            nc.vector.tensor_mul(D[:], ps[:], sb["diagmask"][:])
            tps = tr_psum.tile([4, 480], FP32, tag="tr")
            nc.tensor.matmul(tps[:], sb["blockind"][:], D[:], start=True, stop=True)
            tr = work.tile([4, 16], FP32, tag="tr16")
            nc.vector.reduce_sum(tr[:], mk(tps, 0, [[ROW, 16], [1, ROW]]), axis=AX.X)
            recip = work.tile([4, 16], FP32, tag="recip")
            nc.vector.reciprocal(recip[:], tr[:])
            rbp = tr_psum.tile([120, 16], FP32, tag="tr")
            nc.tensor.matmul(rbp[:], sb["bindT"][:], recip[:], start=True, stop=True)
            rb = work.tile([120, 16], FP32, tag="rb")
            nc.vector.tensor_copy(rb[:], rbp[:])
            Z = work.tile([120, 480], FP32, tag="Z")
            nc.vector.tensor_mul(Z[:], ps[:], mk(rb, 0, [[1, 16], [0, ROW]]))
            Zsq = work.tile([120, 480], FP32, tag="Zsq")
            nc.vector.tensor_mul(Zsq[:], Z[:], Z[:])
            zperm = mk(Z, 0, [[1, ROW], [ROW, 16]])
            zsqperm = mk(Zsq, 0, [[1, ROW], [ROW, 16]])
            if g == 0:
                nc.vector.reduce_sum(acc1[:], zperm, axis=AX.X)
                nc.vector.reduce_sum(acc2[:], zsqperm, axis=AX.X)
                nc.sync.dma_start(z0keep[:], Z[90:120, 0:ROW])
            else:
                t1 = work.tile([120, ROW], FP32, tag="redtmp")
                nc.vector.reduce_sum(t1[:], zperm, axis=AX.X)
                nc.vector.tensor_add(acc1[:], acc1[:], t1[:])
                t2 = work.tile([120, ROW], FP32, tag="redtmp2")
                nc.vector.reduce_sum(t2[:], zsqperm, axis=AX.X)
                nc.vector.tensor_add(acc2[:], acc2[:], t2[:])

        # ---- fold delta1 blocks + payload [30, 120] = [A1f | A2f | mZ0 | mZ0sq]
        payload = accs.tile([ROW, 120], FP32, tag="payload")
        ftmp = accs.tile([ROW, 6 * ROW], FP32, tag="ftmp")
        for ai, (acc, col) in enumerate(((acc1, 0), (acc2, ROW))):
            for b in range(3):
                nc.sync.dma_start(
                    ftmp[:, (ai * 3 + b) * ROW:(ai * 3 + b + 1) * ROW],
                    acc[30 * (b + 1):30 * (b + 2), :])
            dst = payload[:, col:col + ROW]
            nc.vector.tensor_add(dst, acc[0:30, :],
                                 ftmp[:, ai * 3 * ROW:(ai * 3 + 1) * ROW])
            nc.vector.tensor_add(dst, dst, ftmp[:, (ai * 3 + 1) * ROW:(ai * 3 + 2) * ROW])
            nc.vector.tensor_add(dst, dst, ftmp[:, (ai * 3 + 2) * ROW:(ai * 3 + 3) * ROW])
        nc.vector.tensor_scalar_mul(payload[:, 60:90], z0keep[:], sb["c0mask"][:])
        nc.vector.scalar_tensor_tensor(payload[:, 90:120], z0keep[:], sb["c0mask"][:],
                                       z0keep[:], op0=AT.mult, op1=AT.mult)

        # ---- AllReduce
        cc_in = dram.tile([ROW, 120], FP32, tag="ccin")
        cc_out = dram.tile([ROW, 120], FP32, tag="ccout")
        nc.sync.dma_start(cc_in[:], payload[:])
        nc.gpsimd.collective_compute(
            "AllReduce", AT.add, replica_groups=[list(range(NC))],
            ins=[cc_in.opt()], outs=[cc_out.opt()])
        res = accs.tile([ROW, 120], FP32, tag="res")
        nc.sync.dma_start(res[:], cc_out[:])

        # ---- final stats
        g1t = head_psum.tile([ROW, ROW], FP32, tag="hps")
        nc.tensor.matmul(g1t[:], res[:, 0:30], sb["ident"][:], start=True, stop=True)
        g2t = head_psum.tile([ROW, ROW], FP32, tag="hps")
        nc.tensor.matmul(g2t[:], res[:, 30:60], sb["ident"][:], start=True, stop=True)
        st1 = headp.tile([ROW, ROW], FP32, tag="st1")
        nc.vector.tensor_add(st1[:], res[:, 0:30], g1t[:])
        nc.vector.tensor_sub(st1[:], st1[:], res[:, 60:90])
        ss = headp.tile([ROW, ROW], FP32, tag="ss")
        nc.vector.tensor_add(ss[:], res[:, 30:60], g2t[:])
        nc.vector.tensor_sub(ss[:], ss[:], res[:, 90:120])
        gm = headp.tile([ROW, ROW], FP32, tag="gm")
        nc.scalar.activation(gm[:], st1[:], AF.Copy, bias=0.5, scale=0.5 / NLAG)
        q = headp.tile([ROW, ROW], FP32, tag="q")
        nc.vector.tensor_mul(q[:], st1[:], st1[:])
        t4 = headp.tile([ROW, ROW], FP32, tag="t4")
        nc.vector.scalar_tensor_tensor(t4[:], q[:], -1.0 / NLAG, ss[:],
                                       op0=AT.mult, op1=AT.add)
        gv = headp.tile([ROW, ROW], FP32, tag="gv")
        nc.scalar.activation(gv[:], t4[:], AF.Copy, bias=0.5, scale=0.5 / (NLAG - 1))


        # ---- conv head
        gpad = headp.tile([32, 64], FP32, tag="gpad")
        nc.vector.memset(gpad[:], 0.0)
        nc.sync.dma_start(gpad[1:31, 1:31], gm[:])
        nc.sync.dma_start(gpad[1:31, 33:63], gv[:])
        im1 = headp.tile([32, 841], FP32, tag="im1")
        for ic in range(2):
            for ky in range(4):
                for kx in range(4):
                    r = ic * 16 + ky * 4 + kx
                    nc.sync.dma_start(
                        im1[r:r + 1, :],
                        gpad[ky:ky + 29, ic * 32 + kx:ic * 32 + kx + 29])
        h1 = headp.tile([8, 841], FP32, tag="h1")
        for lo, hi in ((0, 424), (424, 841)):
            hp = head_psum.tile([8, hi - lo], FP32, tag="hps")
            nc.tensor.matmul(hp[:], sb["cw1"][:], im1[:, lo:hi], start=True, stop=True)
            nc.vector.tensor_scalar_add(h1[:, lo:hi], hp[:], sb["cb1"][:])
            nc.vector.scalar_tensor_tensor(h1[:, lo:hi], h1[:, lo:hi], 0.2,
                                           h1[:, lo:hi], op0=AT.mult, op1=AT.max)
        p1 = headp.tile([8, 9], FP32, tag="p1")
        for py in range(3):
            for px in range(3):
                win = mk(h1, (8 * py) * 29 + 8 * px, [[29, 8], [1, 8]])
                nc.vector.reduce_max(p1[:, py * 3 + px:py * 3 + px + 1], win,
                                     axis=AX.XY)
        pad1 = headp.tile([8, 25], FP32, tag="pad1")
        nc.vector.memset(pad1[:], 0.0)
        nc.sync.dma_start(mk(pad1, 6, [[5, 3], [1, 3]]), p1[:])
        im2 = headp.tile([32, 16], FP32, tag="im2")
        for ky in range(2):
            for kx in range(2):
                b = (ky * 2 + kx) * 8
                nc.sync.dma_start(im2[b:b + 8, :],
                                  mk(pad1, ky * 5 + kx, [[5, 4], [1, 4]]))
        h2p = head_psum.tile([16, 16], FP32, tag="hps")
        nc.tensor.matmul(h2p[:], sb["cw2"][:], im2[:], start=True, stop=True)
        h2 = headp.tile([16, 16], FP32, tag="h2")
        nc.vector.tensor_scalar_add(h2[:], h2p[:], sb["cb2"][:])
        nc.vector.scalar_tensor_tensor(h2[:], h2[:], 0.2, h2[:],
                                       op0=AT.mult, op1=AT.max)
        h3 = headp.tile([16, 1], FP32, tag="h3")
        nc.vector.reduce_max(h3[:], h2[:], axis=AX.X)
        fcp = head_psum.tile([1, 2], FP32, tag="hps")
        nc.tensor.matmul(fcp[:], h3[:], sb["fwt"][:], start=True, stop=True)
        osb = headp.tile([1, 2], FP32, tag="osb")
        nc.vector.tensor_add(osb[:], fcp[:], sb["fb"][:])
        nc.sync.dma_start(out_d[:], osb[:])


# ---------------------------------------------------------------- entrypoint
_NC_CACHE = []


def kernel(**inputs):
    """Full inputs -> full output (1,2) float32. Shards internally across 8 cores."""
    from concourse.bass_utils import run_bass_kernel_spmd
    if not _NC_CACHE:
        _NC_CACHE.append(build_nc())
    nc = _NC_CACHE[0]
    maps = host_inputs(**{k: np.asarray(v) for k, v in inputs.items()})
    res = run_bass_kernel_spmd(nc, maps, core_ids=list(range(NC)))
    return np.asarray(res.results[0]["out"], np.float32)



# revision 6
# speedup vs baseline: 1.9518x; 1.9518x over previous
"""Bass/Tile kernel for nn_CNN2: lagged cross-correlation + mean/var + tiny CNN head.

Sharding: interleaved lags across 8 cores. Core m computes lags
d = 512g + 128b + 32a + 4m + (3-d1), g,b,a,d1 in [0,4).
The 4m offset is baked into per-core input data placement (pxr); the program
is identical across cores (SPMD).

v2: contiguous-run shift copies (d1-major / v-major layouts), grouped W-fold,
matmul-based payload fold, fully on-chip conv head via banded-weight matmuls
with the Gm/Gv affine scaling folded into host-prepared weights (padding
borders become the constants -NLAG / -(NLAG-1)).
"""
import numpy as np
import ml_dtypes

import concourse.bass as bass
import concourse.bacc as bacc
import concourse.tile as tile
from concourse import mybir

bf16 = ml_dtypes.bfloat16
FP32 = mybir.dt.float32
BF16 = mybir.dt.bfloat16

T = 2048
ROW = 30
NC = 8
OL = 4            # frame offset
BM = 128          # buffer lead margin
NCH = 18          # ylb/yrb chunks (incl. zero chunk 0)
WBUF = 128 * NCH  # 2304
NCC = 17          # contraction chunks
NLAG = 2 * T - 1  # 4095

# packf (f32) column layout
PF_FOLDI = 0      # [120, 30]
PF_SELI = 30      # [120, 30]
PF_ID30 = 60      # [30, 30]
PF_C0 = 90        # [30, 1]
PF_FWT = 91       # [16, 2]
PF_FB = 93        # [1, 2]
PF_CB2 = 95       # [64, 1]
PF_BPL = 96       # [29, 232]
PF_W64 = 328      # 4 x [64, 232]
PF_W2 = 1256      # 2 x [24, 64]
PF_ID32 = 1384    # [32, 32]
PF_GMGV = 1416    # [30, 64] border init
PF_GPAD = 1480    # [32, 64] border init
PF_COLS = 1544

INPUT_SPECS = [
    ("pxl", [ROW, WBUF], BF16), ("pxr", [ROW, WBUF], BF16),
    ("wt", [ROW, ROW], BF16),
    ("blockind", [120, 4], BF16), ("diagmask", [120, 480], FP32),
    ("bindT", [4, 120], FP32),
    ("packf", [128, PF_COLS], FP32),
]


# ---------------------------------------------------------------- host prep
def host_inputs(x, W, conv1_w, conv1_b, conv2_w, conv2_b, fc_w, fc_b):
    """Returns per-core input maps (program constants + per-core data)."""
    x = np.asarray(x, np.float32)
    W = np.asarray(W, np.float32)
    conv1_w = np.asarray(conv1_w, np.float32)
    conv1_b = np.asarray(conv1_b, np.float32)
    conv2_w = np.asarray(conv2_w, np.float32)
    conv2_b = np.asarray(conv2_b, np.float32)
    fc_w = np.asarray(fc_w, np.float32)
    fc_b = np.asarray(fc_b, np.float32)

    def px(off):
        p = np.zeros((ROW, WBUF), bf16)
        w0 = BM + off
        n = min(T, WBUF - w0)
        p[:, w0:w0 + n] = x[:, :n].astype(bf16)
        return p

    blockind = np.zeros((120, 4), bf16)
    diagmask = np.zeros((120, 480), np.float32)
    for d1 in range(4):
        blockind[d1 * 30:d1 * 30 + 30, d1] = 1.0
        for b2 in range(16):
            for i in range(ROW):
                diagmask[d1 * 30 + i, b2 * 30 + i] = 1.0
    bindT = np.ascontiguousarray(blockind.astype(np.float32).T)

    # ---- packf (f32)
    packf = np.zeros((128, PF_COLS), np.float32)
    for d1 in range(4):
        packf[d1 * 30:(d1 + 1) * 30, PF_FOLDI:PF_FOLDI + 30] = np.eye(30)
    packf[90:120, PF_SELI:PF_SELI + 30] = np.eye(30)
    packf[0:30, PF_ID30:PF_ID30 + 30] = np.eye(30)
    packf[0:16, PF_FWT:PF_FWT + 2] = fc_w.T
    packf[0:1, PF_FB:PF_FB + 2] = fc_b.reshape(1, 2)
    packf[0:64, PF_CB2] = np.repeat(conv2_b, 4)
    b1p = conv1_b + 0.5 * conv1_w.sum(axis=(1, 2, 3))
    packf[0:29, PF_BPL:PF_BPL + 232] = np.tile(
        b1p[None, :, None], (29, 1, 29)).reshape(29, 232)

    a_ic = np.array([0.5 / NLAG, 0.5 / (NLAG - 1)], np.float32)
    w1s = conv1_w * a_ic[None, :, None, None]
    for ky in range(4):
        blk = np.zeros((64, 232), np.float32)
        for ic in range(2):
            for xp in range(32):
                for xx in range(29):
                    kx = xp - xx
                    if 0 <= kx < 4:
                        for oc in range(8):
                            blk[ic * 32 + xp, oc * 29 + xx] = w1s[oc, ic, ky, kx]
        packf[0:64, PF_W64 + 232 * ky:PF_W64 + 232 * (ky + 1)] = blk
    for ky in range(2):
        blk = np.zeros((24, 64), np.float32)
        for c8 in range(8):
            for px_ in range(3):
                for oc2 in range(16):
                    for x2 in range(4):
                        kx = px_ - x2 + 1
                        if 0 <= kx < 2:
                            blk[c8 * 3 + px_, oc2 * 4 + x2] = conv2_w[oc2, c8, ky, kx]
        packf[0:24, PF_W2 + 64 * ky:PF_W2 + 64 * (ky + 1)] = blk
    packf[0:32, PF_ID32:PF_ID32 + 32] = np.eye(32)
    gmgv = np.zeros((30, 64), np.float32)
    gmgv[:, 0] = gmgv[:, 31] = -float(NLAG)
    gmgv[:, 32] = gmgv[:, 63] = -float(NLAG - 1)
    packf[0:30, PF_GMGV:PF_GMGV + 64] = gmgv
    gpad = np.zeros((32, 64), np.float32)
    gpad[0, 0:32] = gpad[31, 0:32] = -float(NLAG)
    gpad[0, 32:64] = gpad[31, 32:64] = -float(NLAG - 1)
    packf[0:32, PF_GPAD:PF_GPAD + 64] = gpad

    shared = {
        "pxl": px(OL),
        "wt": np.ascontiguousarray(W.T).astype(bf16),   # rhs[k,i] = W[i,k]
        "blockind": blockind,
        "diagmask": diagmask,
        "bindT": bindT,
    }
    per_core = []
    for m in range(NC):
        d = dict(shared)
        d["pxr"] = px(OL + 4 * m)
        pf = packf.copy()
        pf[0:30, PF_C0] = 1.0 if m == 0 else 0.0
        d["packf"] = pf
        per_core.append(d)
    return per_core


def mk(t, off, dims):
    """AP on tile t with explicit free dims; partition pair preserved."""
    ap = t[:]
    return bass.AP(ap.tensor, ap.offset + off,
                   [list(ap.ap[0])] + [list(d) for d in dims])


# ---------------------------------------------------------------- kernel
def build_nc():
    nc = bacc.Bacc("TRN2", target_bir_lowering=False, debug=False, num_devices=NC)
    din = {}
    for name, shape, dt in INPUT_SPECS:
        din[name] = nc.dram_tensor(name, shape, dt, kind="ExternalInput").ap()
    out_d = nc.dram_tensor("out", [1, 2], FP32, kind="ExternalOutput").ap()
    with tile.TileContext(nc) as tc:
        _body(tc, din, out_d)
    nc.compile()
    return nc


def _body(tc, din, out_d):
    nc = tc.nc
    AT = mybir.AluOpType
    AX = mybir.AxisListType
    AF = mybir.ActivationFunctionType

    from contextlib import ExitStack
    ctx = ExitStack()
    with ctx:
        consts = ctx.enter_context(tc.tile_pool(name="consts", bufs=1))
        copies_p = ctx.enter_context(tc.tile_pool(name="copies", bufs=1))
        work = ctx.enter_context(tc.tile_pool(name="work", bufs=2))
        accs_p = ctx.enter_context(tc.tile_pool(name="accs", bufs=1))
        dram = ctx.enter_context(tc.tile_pool(name="dram", bufs=1, space="DRAM"))
        headp = ctx.enter_context(tc.tile_pool(name="head", bufs=1))

        # ---- load inputs to SBUF
        sb = {}
        for name, shape, dt in INPUT_SPECS:
            t = consts.tile(shape, dt, tag=name)
            nc.sync.dma_start(t[:], din[name][:])
            sb[name] = t
        packf = sb["packf"]
        foldI = packf[0:120, PF_FOLDI:PF_FOLDI + 30]
        selI = packf[0:120, PF_SELI:PF_SELI + 30]
        id30 = packf[0:30, PF_ID30:PF_ID30 + 30]
        c0mask = packf[0:30, PF_C0:PF_C0 + 1]
        fwt = packf[0:16, PF_FWT:PF_FWT + 2]
        fb = packf[0:1, PF_FB:PF_FB + 2]
        cb2rep = packf[0:64, PF_CB2:PF_CB2 + 1]
        bplane = packf[0:29, PF_BPL:PF_BPL + 232]
        id32 = packf[0:32, PF_ID32:PF_ID32 + 32]

        # head working tiles (init off critical path)
        gmgv = headp.tile([30, 64], FP32, tag="gmgv")
        gpad = headp.tile([32, 64], FP32, tag="gpad")
        p1T5 = headp.tile([24, 5], FP32, tag="p1T5")
        nc.sync.dma_start(gmgv[:], packf[0:30, PF_GMGV:PF_GMGV + 64])
        nc.sync.dma_start(gpad[:], packf[0:32, PF_GPAD:PF_GPAD + 64])
        nc.vector.memset(p1T5[:], 0.0)

        # ---- shift-folded W-fold: the d1 / (3+32v) time shifts are column
        # offsets into pxl/pxr, so each psum chunk lands directly in the
        # interleaved layout the main matmuls need. No shift copies at all.
        # ylc[p, (c, d1, i)] = yl[i, 128(c+1) + p - d1]
        # yrc[p, (cc, v, i)] = yr[i, 128(cc-2) + p - 3 - 32v]  (cc>=3; 0..2 zero)
        ylc = copies_p.tile([128, NCC * 120], BF16, tag="ylc")
        yrc = copies_p.tile([128, (3 + NCC) * 120], BF16, tag="yrc")
        nc.vector.memset(yrc[:, 0:360], 0.0)
        with tc.tile_pool(name="wfpsum", bufs=2, space="PSUM") as wf_psum:
            for c in range(NCC):
                ps = wf_psum.tile([128, 120], FP32, tag="wf")
                for d1 in range(4):
                    off = 128 * (c + 1) - d1
                    nc.tensor.matmul(ps[:, 30 * d1:30 * d1 + 30],
                                     sb["pxl"][:, off:off + 128], sb["wt"][:],
                                     start=True, stop=True)
                nc.vector.tensor_copy(ylc[:, 120 * c:120 * (c + 1)], ps[:])
            for cc in range(3, 3 + NCC):
                ps = wf_psum.tile([128, 120], FP32, tag="wf")
                for v in range(4):
                    off = 128 * (cc - 2) - 3 - 32 * v
                    nc.tensor.matmul(ps[:, 30 * v:30 * v + 30],
                                     sb["pxr"][:, off:off + 128], sb["wt"][:],
                                     start=True, stop=True)
                nc.vector.tensor_copy(yrc[:, 120 * cc:120 * (cc + 1)], ps[:])

        # ---- main matmul groups + trace + scaled accumulation
        accs = accs_p.tile([120, 60], FP32, tag="accs")
        payload = accs_p.tile([ROW, 120], FP32, tag="payload")

        with tc.tile_pool(name="mmpsum", bufs=2, space="PSUM") as mm_psum, \
             tc.tile_pool(name="trpsum", bufs=2, space="PSUM") as tr_psum, \
             tc.tile_pool(name="auxpsum", bufs=1, space="PSUM") as aux_psum:
            for g in range(4):
                ps = mm_psum.tile([120, 480], FP32, tag="mm")
                for c in range(4 * g, NCC):
                    lhsT = mk(ylc, 120 * c, [[1, 120]])
                    rhs = mk(yrc, 120 * (3 + c - 4 * g), [[30, 4], [-120, 4], [1, 30]])
                    nc.tensor.matmul(ps[:], lhsT, rhs,
                                     start=(c == 4 * g), stop=(c == NCC - 1))
                # traces: mask diag, column-fold via matmul, reduce, reciprocal
                D = work.tile([120, 480], BF16, tag="D")
                nc.vector.tensor_mul(D[:], ps[:], sb["diagmask"][:])
                tps = tr_psum.tile([4, 480], FP32, tag="tr")
                nc.tensor.matmul(tps[:], sb["blockind"][:], D[:],
                                 start=True, stop=True)
                tr = work.tile([4, 16], FP32, tag="tr16")
                nc.vector.reduce_sum(tr[:], mk(tps, 0, [[ROW, 16], [1, ROW]]),
                                     axis=AX.X)
                recip = work.tile([4, 16], FP32, tag="recip")
                nc.vector.reciprocal(recip[:], tr[:])
                rbp = tr_psum.tile([120, 16], FP32, tag="tr")
                nc.tensor.matmul(rbp[:], sb["bindT"][:], recip[:],
                                 start=True, stop=True)
                rb = work.tile([120, 16], FP32, tag="rb")
                nc.vector.tensor_copy(rb[:], rbp[:])
                Z = work.tile([120, 480], FP32, tag="Z")
                nc.vector.tensor_mul(Z[:], ps[:], mk(rb, 0, [[1, 16], [0, ROW]]))
                Zsq = work.tile([120, 480], FP32, tag="Zsq")
                nc.scalar.activation(Zsq[:], Z[:], AF.Square)
                zperm = mk(Z, 0, [[1, ROW], [ROW, 16]])
                zsqperm = mk(Zsq, 0, [[1, ROW], [ROW, 16]])
                if g == 0:
                    nc.vector.reduce_sum(accs[:, 0:30], zperm, axis=AX.X)
                    nc.vector.reduce_sum(accs[:, 30:60], zsqperm, axis=AX.X)
                    z0p = aux_psum.tile([ROW, ROW], FP32, tag="aux")
                    nc.tensor.matmul(z0p[:], selI, Z[:, 0:30], start=True, stop=True)
                    nc.vector.tensor_scalar_mul(payload[:, 60:90], z0p[:], c0mask)
                    nc.vector.tensor_mul(payload[:, 90:120],
                                         payload[:, 60:90], z0p[:])
                else:
                    t1 = work.tile([120, ROW], FP32, tag="redtmp")
                    nc.vector.reduce_sum(t1[:], zperm, axis=AX.X)
                    nc.vector.tensor_add(accs[:, 0:30], accs[:, 0:30], t1[:])
                    t2 = work.tile([120, ROW], FP32, tag="redtmp2")
                    nc.vector.reduce_sum(t2[:], zsqperm, axis=AX.X)
                    nc.vector.tensor_add(accs[:, 30:60], accs[:, 30:60], t2[:])

            # ---- fold delta1 blocks: payload[:, 0:60] = foldI.T @ accs
            foldp = aux_psum.tile([ROW, 60], FP32, tag="aux")
            nc.tensor.matmul(foldp[:], foldI, accs[:], start=True, stop=True)
            nc.vector.tensor_copy(payload[:, 0:60], foldp[:])

        # ---- AllReduce
        cc_in = dram.tile([ROW, 120], FP32, tag="ccin")
        cc_out = dram.tile([ROW, 120], FP32, tag="ccout")
        nc.sync.dma_start(cc_in[:], payload[:])
        nc.gpsimd.collective_compute(
            "AllReduce", AT.add, replica_groups=[list(range(NC))],
            ins=[cc_in.opt()], outs=[cc_out.opt()])
        res = accs_p.tile([ROW, 120], FP32, tag="res")
        nc.sync.dma_start(res[:], cc_out[:])

        # ---- final stats + conv head (fully on-chip)
        with tc.tile_pool(name="headpsum", bufs=2, space="PSUM") as head_psum:
            g1t = head_psum.tile([ROW, ROW], FP32, tag="hps")
            nc.tensor.matmul(g1t[:], res[:, 0:30], id30, start=True, stop=True)
            g2t = head_psum.tile([ROW, ROW], FP32, tag="hps")
            nc.tensor.matmul(g2t[:], res[:, 30:60], id30, start=True, stop=True)
            st1 = headp.tile([ROW, ROW], FP32, tag="st1")
            nc.vector.tensor_add(st1[:], res[:, 0:30], g1t[:])
            nc.vector.tensor_sub(st1[:], st1[:], res[:, 60:90])
            ss = headp.tile([ROW, ROW], FP32, tag="ss")
            nc.vector.tensor_add(ss[:], res[:, 30:60], g2t[:])
            nc.vector.tensor_sub(ss[:], ss[:], res[:, 90:120])
            q = headp.tile([ROW, ROW], FP32, tag="q")
            nc.vector.tensor_mul(q[:], st1[:], st1[:])
            t4 = headp.tile([ROW, ROW], FP32, tag="t4")
            nc.vector.scalar_tensor_tensor(t4[:], q[:], -1.0 / NLAG, ss[:],
                                           op0=AT.mult, op1=AT.add)
            # raw st1/t4 planes into gmgv (scaling folded into conv1 weights)
            nc.vector.tensor_copy(gmgv[:, 1:31], st1[:])
            nc.vector.tensor_copy(gmgv[:, 33:63], t4[:])
            nc.sync.dma_start(gpad[1:31, 0:64], gmgv[:])

            # transpose gpad -> gpadT [64(ic,xp), 32(y)]
            gpT = head_psum.tile([64, 32], FP32, tag="hps")
            nc.tensor.matmul(gpT[:], gpad[:], id32, start=True, stop=True)
            gpadT = headp.tile([64, 32], FP32, tag="gpadT")
            nc.vector.tensor_copy(gpadT[:], gpT[:])

            # conv1 as 4 banded matmuls: h1p[y, (oc,x)] (29, 232)
            h1p = head_psum.tile([29, 232], FP32, tag="hps")
            for ky in range(4):
                nc.tensor.matmul(
                    h1p[:], gpadT[:, ky:ky + 29],
                    packf[0:64, PF_W64 + 232 * ky:PF_W64 + 232 * (ky + 1)],
                    start=(ky == 0), stop=(ky == 3))
            h1s = headp.tile([29, 232], FP32, tag="h1s")
            nc.vector.tensor_add(h1s[:], h1p[:], bplane)
            nc.vector.scalar_tensor_tensor(h1s[:], h1s[:], 0.2, h1s[:],
                                           op0=AT.mult, op1=AT.max)
            # maxpool 8x8: pool-x (free dim), transpose, pool-y
            px1 = headp.tile([29, 24], FP32, tag="px1")
            nc.vector.reduce_max(px1[:], mk(h1s, 0, [[29, 8], [8, 3], [1, 8]]),
                                 axis=AX.X)
            t1ps = head_psum.tile([24, 29], FP32, tag="hps")
            nc.tensor.matmul(t1ps[:], px1[:], id32[0:29, 0:29],
                             start=True, stop=True)
            nc.vector.reduce_max(p1T5[:, 1:4], mk(t1ps, 0, [[8, 3], [1, 8]]),
                                 axis=AX.X)
            # conv2 as 2 banded matmuls: h2p[(oc2,x2), y2] (64, 4)
            h2p = head_psum.tile([64, 4], FP32, tag="hps")
            for ky in range(2):
                nc.tensor.matmul(
                    h2p[:], packf[0:24, PF_W2 + 64 * ky:PF_W2 + 64 * (ky + 1)],
                    p1T5[:, ky:ky + 4], start=(ky == 0), stop=(ky == 1))
            h2s = headp.tile([64, 4], FP32, tag="h2s")
            nc.vector.tensor_scalar_add(h2s[:], h2p[:], cb2rep)
            nc.vector.scalar_tensor_tensor(h2s[:], h2s[:], 0.2, h2s[:],
                                           op0=AT.mult, op1=AT.max)
            # maxpool 4x4 (global): reduce y2, regroup, reduce x2
            h2r = headp.tile([64, 1], FP32, tag="h2r")
            nc.vector.reduce_max(h2r[:], h2s[:], axis=AX.X)
            h3m = headp.tile([16, 4], FP32, tag="h3m")
            nc.sync.dma_start(h3m[:], h2r[:])
            h3 = headp.tile([16, 1], FP32, tag="h3")
            nc.vector.reduce_max(h3[:], h3m[:], axis=AX.X)
            fcp = head_psum.tile([1, 2], FP32, tag="hps")
            nc.tensor.matmul(fcp[:], h3[:], fwt, start=True, stop=True)
            osb = headp.tile([1, 2], FP32, tag="osb")
            nc.vector.tensor_add(osb[:], fcp[:], fb)
            nc.sync.dma_start(out_d[:], osb[:])


# ---------------------------------------------------------------- entrypoint
_NC_CACHE = []


def kernel(**inputs):
    """Full inputs -> full output (1,2) float32. Shards internally across 8 cores."""
    from concourse.bass_utils import run_bass_kernel_spmd
    if not _NC_CACHE:
        _NC_CACHE.append(build_nc())
    nc = _NC_CACHE[0]
    maps = host_inputs(**{k: np.asarray(v) for k, v in inputs.items()})
    res = run_bass_kernel_spmd(nc, maps, core_ids=list(range(NC)))
    return np.asarray(res.results[0]["out"], np.float32)
